# revision 30
# baseline (speedup 1.0000x reference)
# GAT (graph attention) Trainium2 kernel — 8-core row-parallel SPMD,
# pipelined: one prologue program + row-chunk programs overlapping
# upload / exec / download on the slow axon tunnel.
#
# Math (per head h, rows I owned by a core):
#   h = x @ W_h ; f1 = h@a1 ; f2 = h@a2 ; z_ij = f1_i + f2_j
#   P_ij = adj_ij ? exp(lrelu(z)) : exp(9e-15 ~= 0) ; att = softmax_j(P)
#   out = elu( (P @ h) / (P @ 1) )
# Device factorization (avoids O(N^2) transcendentals):
#   exp(lrelu(z)) = u'_i * v'_j * max(r_i * w_j, 1)
#     r = e^{0.8 f1}, w = e^{0.8 f2}, u' = e^{0.2 f1}, v' = e^{0.2 f2}
#   E2[j,i] = m^T[j,i] * max(r_i * (w_j v'_j), v'_j)     (ts_dual + tt mult)
#   numer[d,i] = u'_i * ([h|1]^T E2)[d,i] + S[d] - (h^T m^T)[d,i]
#   denom[i]   = u'_i * Y1[i] + N - deg_i
#
# The e2e bottleneck is the axon RPC tunnel: ~80ms RTT, ~25-45MB/s shared
# bandwidth, one host CPU. Design:
#   - adj ships BIT-PACKED along j (uint8, 32x less wire), packed by a
#     gcc-compiled AVX2 helper (~22ms for all 256MB vs ~174ms numpy).
#   - x (bf16), W (bf16), a (f32) ship once as per-core slices in one
#     combined buffer (~2.1MB); progA all-gathers x and precomputes every
#     j-side quantity (h~, e^{f2} factors) into device-DRAM blobs.
#   - the row space is split into row-chunks; each chunk is a light progB
#     variant (row offset baked in, no collectives) dispatched as soon as
#     its adj chunk is on the wire. Chunk execs and int8 output downloads
#     overlap the upload of later chunks; only the last chunk's small
#     exec+fetch tail is exposed.
#   - output returns int8 with per-row f32 scales bitcast into tail rows;
#     host dequantizes to f32.

import ctypes
import os
import subprocess
import tempfile

import numpy as np

N = 8192
EMB = 128
HID = 64
NH = 4
NCORES = 8
NBLK = N // NCORES        # 1024 rows per core
NP8 = N // 8              # packed bytes per adj row
NJC = N // 128            # j chunks

# (rows, row offset) per chunk, per core; rows must be multiples of 128.
# Descending sizes: the last chunk's exec + download is the exposed tail.
CHUNKS = [(384, 0), (256, 384), (256, 640), (128, 896)]

# Per-core xwa segment layout:
#   [ x rows u8 (int8+128, per-col scale) | colscale/bias f32 [128,2]
#   | F rows f32 [NBLK,8] (f1/f2 per head, computed EXACTLY on host)
#   | W eighth bf16 | a eighth f32 ]
# x int8 + exact F is both SMALLER (169KB vs 264KB per core) and MORE
# ACCURATE than bf16 x: the error-critical exp(f) path uses exact f32
# logits, and the int8 h error washes out in the softmax average.
XB = NBLK * EMB           # x rows, 1 byte each
CSB = 128 * 2 * 4         # per-column scale+bias f32
FB = NBLK * 2 * NH * 4    # F rows f32
WB8 = NH * EMB * HID * 2 // NCORES
SEG = XB + CSB + FB + WB8
OFF_CS = XB
OFF_F = OFF_CS + CSB
OFF_W = OFF_F + FB

_cache = {}

_PACK_C = r"""
#include <stdint.h>
#include <stddef.h>
#if defined(__AVX2__)
#include <immintrin.h>
#endif

void pack_adj(const int32_t* restrict adj, uint8_t* restrict out,
              size_t nrows, size_t ncols) {
    size_t nb = ncols / 8;
    for (size_t r = 0; r < nrows; r++) {
        const int32_t* row = adj + r * ncols;
        uint8_t* orow = out + r * nb;
        size_t c = 0;
#if defined(__AVX2__)
        for (; c + 8 <= ncols; c += 8) {
            __m256i v = _mm256_loadu_si256((const __m256i*)(row + c));
            v = _mm256_slli_epi32(v, 31);
            int m = _mm256_movemask_ps(_mm256_castsi256_ps(v));
            orow[c / 8] = (uint8_t)m;
        }
#endif
        for (; c < ncols; c += 8) {
            uint8_t b = 0;
            for (int k = 0; k < 8; k++)
                b |= (uint8_t)((row[c + k] & 1) << k);
            orow[c / 8] = b;
        }
    }
}

/* column-wise absmax of a row-major [nrows, 128] f32 matrix */
void colmax_abs(const float* restrict x, float* restrict cmax,
                size_t nrows) {
    for (int e = 0; e < 128; e++) cmax[e] = 0.0f;
#if defined(__AVX2__)
    __m256 acc[16];
    for (int v = 0; v < 16; v++) acc[v] = _mm256_setzero_ps();
    const __m256 signmask = _mm256_castsi256_ps(_mm256_set1_epi32(0x7fffffff));
    for (size_t r = 0; r < nrows; r++) {
        const float* row = x + r * 128;
        for (int v = 0; v < 16; v++) {
            __m256 d = _mm256_and_ps(_mm256_loadu_ps(row + v * 8), signmask);
            acc[v] = _mm256_max_ps(acc[v], d);
        }
    }
    for (int v = 0; v < 16; v++) _mm256_storeu_ps(cmax + v * 8, acc[v]);
#else
    for (size_t r = 0; r < nrows; r++)
        for (int e = 0; e < 128; e++) {
            float d = x[r * 128 + e];
            if (d < 0) d = -d;
            if (d > cmax[e]) cmax[e] = d;
        }
#endif
}

/* quantize row-major [nrows, 128] f32 -> biased u8 with per-col 1/scale */
void quant_x(const float* restrict x, const float* restrict invscale,
             uint8_t* restrict out, size_t nrows) {
#if defined(__AVX2__)
    __m256 inv[16];
    for (int v = 0; v < 16; v++) inv[v] = _mm256_loadu_ps(invscale + v * 8);
    const __m256i bias = _mm256_set1_epi32(128);
    for (size_t r = 0; r < nrows; r++) {
        const float* row = x + r * 128;
        uint8_t* orow = out + r * 128;
        for (int v = 0; v < 16; v += 4) {
            __m256i q0 = _mm256_add_epi32(_mm256_cvtps_epi32(
                _mm256_mul_ps(_mm256_loadu_ps(row + v * 8), inv[v])), bias);
            __m256i q1 = _mm256_add_epi32(_mm256_cvtps_epi32(
                _mm256_mul_ps(_mm256_loadu_ps(row + v * 8 + 8), inv[v + 1])), bias);
            __m256i q2 = _mm256_add_epi32(_mm256_cvtps_epi32(
                _mm256_mul_ps(_mm256_loadu_ps(row + v * 8 + 16), inv[v + 2])), bias);
            __m256i q3 = _mm256_add_epi32(_mm256_cvtps_epi32(
                _mm256_mul_ps(_mm256_loadu_ps(row + v * 8 + 24), inv[v + 3])), bias);
            /* 32 -> 16 -> 8 with saturation; repair 128-bit lane order */
            __m256i p01 = _mm256_packs_epi32(q0, q1);
            __m256i p23 = _mm256_packs_epi32(q2, q3);
            __m256i p = _mm256_packus_epi16(p01, p23);
            p = _mm256_permutevar8x32_epi32(
                p, _mm256_setr_epi32(0, 4, 1, 5, 2, 6, 3, 7));
            _mm256_storeu_si256((__m256i*)(orow + v * 8), p);
        }
    }
#else
    for (size_t r = 0; r < nrows; r++)
        for (int e = 0; e < 128; e++) {
            float q = x[r * 128 + e] * invscale[e];
            long iq = (long)(q >= 0 ? q + 0.5f : q - 0.5f) + 128;
            if (iq < 0) iq = 0;
            if (iq > 255) iq = 255;
            out[r * 128 + e] = (uint8_t)iq;
        }
#endif
}
"""


def _get_packer():
    """Compile the C bit-packer once; fall back to numpy packbits."""
    if "packer" in _cache:
        return _cache["packer"]
    try:
        d = os.path.join(tempfile.gettempdir(), "gat_pack_v2")
        so = os.path.join(d, "pack.so")
        if not os.path.exists(so):
            os.makedirs(d, exist_ok=True)
            src = os.path.join(d, "pack.c")
            with open(src, "w") as f:
                f.write(_PACK_C)
            subprocess.run(
                ["gcc", "-O3", "-march=native", "-shared", "-fPIC",
                 "-o", so + ".tmp", src],
                check=True, capture_output=True)
            os.replace(so + ".tmp", so)
        lib = ctypes.CDLL(so)
        lib.pack_adj.argtypes = [ctypes.c_void_p, ctypes.c_void_p,
                                 ctypes.c_size_t, ctypes.c_size_t]
        lib.pack_adj.restype = None
        lib.colmax_abs.argtypes = [ctypes.c_void_p, ctypes.c_void_p,
                                   ctypes.c_size_t]
        lib.colmax_abs.restype = None
        lib.quant_x.argtypes = [ctypes.c_void_p, ctypes.c_void_p,
                                ctypes.c_void_p, ctypes.c_size_t]
        lib.quant_x.restype = None

        # sanity checks against numpy
        test = np.random.default_rng(0).integers(0, 2, (4, 64), dtype=np.int32)
        got = np.empty((4, 8), np.uint8)
        lib.pack_adj(test.ctypes.data, got.ctypes.data, 4, 64)
        ref = np.packbits(test.astype(np.uint8), axis=1, bitorder="little")
        if not np.array_equal(got, ref):
            raise RuntimeError("C packer mismatch")
        tx = np.random.default_rng(1).normal(size=(16, 128)).astype(np.float32)
        cm = np.empty(128, np.float32)
        lib.colmax_abs(tx.ctypes.data, cm.ctypes.data, 16)
        if not np.allclose(cm, np.abs(tx).max(axis=0)):
            raise RuntimeError("C colmax mismatch")
        inv = (127.0 / np.maximum(cm, 1e-30)).astype(np.float32)
        qu = np.empty((16, 128), np.uint8)
        lib.quant_x(tx.ctypes.data, inv.ctypes.data, qu.ctypes.data, 16)
        qref = (np.clip(np.rint(tx * inv), -127, 127) + 128.0).astype(np.uint8)
        if np.abs(qu.astype(np.int32) - qref.astype(np.int32)).max() > 1:
            raise RuntimeError("C quant mismatch")

        def fn(adj_c, rowstart, nrows, dst):
            # adj_c: contiguous int32 [N, N]; dst: contiguous u8 [nrows, N/8]
            lib.pack_adj(adj_c.ctypes.data + rowstart * N * 4,
                         dst.ctypes.data, nrows, N)

        def quant(x_c):
            cmax = np.empty(128, np.float32)
            lib.colmax_abs(x_c.ctypes.data, cmax.ctypes.data, N)
            scale = np.maximum(cmax, 1e-30).astype(np.float32) / 127.0
            inv = (1.0 / scale).astype(np.float32)
            out = np.empty((N, EMB), np.uint8)
            lib.quant_x(x_c.ctypes.data, inv.ctypes.data, out.ctypes.data, N)
            return out, scale
    except Exception:
        def fn(adj_c, rowstart, nrows, dst):
            blk = adj_c.view(np.uint8)[rowstart:rowstart + nrows, ::4]
            dst[:] = np.packbits(blk, axis=1, bitorder="little")

        def quant(x_c):
            cmax = np.abs(x_c).max(axis=0)
            scale = np.maximum(cmax, 1e-30).astype(np.float32) / 127.0
            out = (np.clip(np.rint(x_c * (1.0 / scale)), -127, 127)
                   + 128.0).astype(np.uint8)
            return out, scale
    _cache["packer"] = (fn, quant)
    return _cache["packer"]


def _mybir():
    import concourse.bass as bass
    import concourse.bacc as bacc
    import concourse.tile as tile
    import concourse.mybir as mybir
    return bass, bacc, tile, mybir


def build_prologue(n=N):
    """progA: all-gather x/W/a, precompute all j-side quantities into
    device-DRAM blobs consumed by the chunk programs."""
    bass, bacc, tile, mybir = _mybir()
    from concourse.masks import make_identity

    fp32 = mybir.dt.float32
    bf16 = mybir.dt.bfloat16
    u8 = mybir.dt.uint8
    Alu = mybir.AluOpType
    Act = mybir.ActivationFunctionType
    MS = bass.MemorySpace

    njc = NJC
    nc = bacc.Bacc(num_devices=NCORES)
    xwa_d = nc.declare_dram_parameter("xwa_blk", [SEG], u8, isOutput=False)
    # outputs: j-side blobs (device-resident, never fetched to host)
    hsb_o = nc.declare_dram_parameter(
        "hsb_o", [128, njc * NH * (HID + 1)], bf16, isOutput=True)
    hpair_o = nc.declare_dram_parameter(
        "hpair_o", [128, njc * NH * HID], bf16, isOutput=True)
    # small_o packs [ETc | Vc] along the free dim
    small_o = nc.declare_dram_parameter(
        "small_o", [128, njc * NH * 2], fp32, isOutput=True)
    srow_o = nc.declare_dram_parameter(
        "srow_o", [1, NH * HID], fp32, isOutput=True)
    # own F rows passed through for the chunk programs' i-side
    fown_o = nc.declare_dram_parameter(
        "fown_o", [NBLK, 2 * NH], fp32, isOutput=True)

    with tile.TileContext(nc) as tc:
        with (
            tc.tile_pool(name="const", bufs=1) as const,
            tc.tile_pool(name="ld", bufs=3) as ld,
            tc.tile_pool(name="dramp", bufs=1, space=MS.DRAM) as dramp,
        ):
            ag_in = dramp.tile([SEG], u8, name="ag_in", tag="ag_in")
            ag_out = dramp.tile([NCORES * SEG], u8, name="ag_out", tag="ag_out",
                                addr_space="Shared")
            nc.sync.dma_start(out=ag_in, in_=xwa_d[:])
            nc.gpsimd.collective_compute(
                "AllGather", Alu.bypass,
                replica_groups=[list(range(NCORES))],
                ins=[ag_in], outs=[ag_out],
            )
            g2 = ag_out.rearrange("(c y) -> c y", c=NCORES)
            # biased-u8 x -> f32 (SWDGE cast); debias+scale happens after the
            # transpose (per-partition affine on xT)
            xag = dramp.tile([n, EMB], fp32, name="xag", tag="xag")
            nc.gpsimd.dma_start(out=xag, in_=g2[:, 0:XB])
            # gathered F rows [n, 8] f32 (row order = core-major = node id)
            Fag = dramp.tile([n, 2 * NH], fp32, name="Fag", tag="Fag")
            nc.sync.dma_start(out=Fag, in_=g2[:, OFF_F:OFF_F + FB].bitcast(fp32))
            # own F rows pass straight through to the chunk programs
            nc.sync.dma_start(
                out=fown_o[:, :],
                in_=xwa_d[OFF_F:OFF_F + FB].bitcast(fp32).rearrange(
                    "(r e) -> r e", e=2 * NH))
            Wg = dramp.tile([NH * EMB * HID], bf16, name="Wg", tag="Wg")
            nc.sync.dma_start(
                out=Wg, in_=g2[:, OFF_W:OFF_W + WB8].bitcast(bf16))
            W_v = Wg.rearrange("(h e d) -> e h d", h=NH, e=EMB)
            # permuted row view: row (b k) of xagv == original row 8k+b == j'
            xagv = xag.rearrange("(k b) e -> b k e", b=8)
            Fagv = Fag.rearrange("(k b) e -> b k e", b=8)

            ident = const.tile([128, 128], fp32, name="ident", tag="ident")
            make_identity(nc, ident)

            # per-column dequant affine: x = u8 * scale + bias
            csb = const.tile([128, 2], fp32, name="csb", tag="csb")
            nc.sync.dma_start(
                out=csb,
                in_=xwa_d[OFF_CS:OFF_CS + CSB].bitcast(fp32).rearrange(
                    "(p t) -> p t", t=2))

            ppsum = tc.alloc_tile_pool(name="ppsum", bufs=2, space=MS.PSUM)
            Wsb = const.tile([128, NH, HID], fp32, name="Wsb", tag="Wsb")
            nc.gpsimd.dma_start(out=Wsb, in_=W_v)

            # x_perm^T  [128e, n], dequantized per partition (= column e)
            xT = const.tile([128, n], fp32, name="xT", tag="xT")
            for jc in range(njc):
                b, m = jc // 8, jc % 8
                xt_nat = ld.tile([128, EMB], fp32, name="xt_nat", tag="xt_nat")
                nc.sync.dma_start(
                    out=xt_nat, in_=xagv[b, m * 128:(m + 1) * 128, :])
                ps = ppsum.tile([128, 128], fp32, name="ps", tag="ps")
                nc.tensor.matmul(ps, xt_nat, ident)
                nc.vector.tensor_scalar(
                    out=xT[:, jc * 128:(jc + 1) * 128], in0=ps,
                    scalar1=csb[:, 0:1], scalar2=csb[:, 1:2],
                    op0=Alu.mult, op1=Alu.add)

            xsum = const.tile([128, 1], fp32, name="xsum", tag="xsum")
            nc.vector.tensor_reduce(xsum, xT, mybir.AxisListType.X, Alu.add)

            Wflat = Wsb.rearrange("e h d -> e (h d)")

            # f columns for all j, loaded from the host-exact gathered F
            # (permuted row order j' like x)
            Fcol = const.tile([128, njc, 2 * NH], fp32, name="Fcol", tag="Fcol")
            for jc in range(njc):
                b, m = jc // 8, jc % 8
                nc.sync.dma_start(
                    out=Fcol[:, jc, :], in_=Fagv[b, m * 128:(m + 1) * 128, :])

            # scalar cols (j side): ETc = e^{f2} (= w v'), Vc = e^{0.2 f2}
            ETc = const.tile([128, njc, NH], fp32, name="ETc", tag="ETc")
            Vc = const.tile([128, njc, NH], fp32, name="Vc", tag="Vc")
            for h in range(NH):
                nc.scalar.activation(ETc[:, :, h], Fcol[:, :, 2 * h + 1], Act.Exp)
                nc.scalar.activation(
                    Vc[:, :, h], Fcol[:, :, 2 * h + 1], Act.Exp, scale=0.2)

            # H~ [128, njc, NH, HID+1] bf16 (ones col at [.., HID]) + pair
            Hsb = const.tile([128, njc, NH, HID + 1], bf16, name="Hsb", tag="Hsb")
            Hpair = const.tile([128, njc, NH * HID], bf16, name="Hpair",
                               tag="Hpair")
            nc.vector.memset(Hsb[:, :, :, HID], 1.0)
            for jc in range(njc):
                h_ps = ppsum.tile([128, NH, HID], fp32, name="h_ps", tag="ps")
                nc.tensor.matmul(
                    h_ps.rearrange("p h d -> p (h d)"),
                    xT[:, jc * 128:(jc + 1) * 128], Wflat)
                nc.scalar.copy(out=Hsb[:, jc, :, 0:HID], in_=h_ps)
                nc.scalar.copy(
                    out=Hpair[:, jc, :].rearrange("p (h d) -> p h d", h=NH),
                    in_=h_ps)

            # S row
            s_ps = ppsum.tile([1, NH * HID], fp32, name="s_ps", tag="ps")
            nc.tensor.matmul(s_ps, xsum, Wflat)
            S_sb = const.tile([1, NH * HID], fp32, name="S_sb", tag="S_sb")
            nc.scalar.copy(out=S_sb, in_=s_ps)
            nc.sync.dma_start(out=srow_o[:, :], in_=S_sb)
            ppsum.release()

            # write blobs
            nc.sync.dma_start(
                out=hsb_o[:, :], in_=Hsb.rearrange("p a b c -> p (a b c)"))
            nc.sync.dma_start(
                out=hpair_o[:, :], in_=Hpair.rearrange("p a b -> p (a b)"))
            nc.sync.dma_start(
                out=small_o[:, 0:njc * NH],
                in_=ETc.rearrange("p a b -> p (a b)"))
            nc.sync.dma_start(
                out=small_o[:, njc * NH:2 * njc * NH],
                in_=Vc.rearrange("p a b -> p (a b)"))
    nc.compile()
    return nc


def build_chunk(nb, off, n=N):
    """progB: i-side work for nb rows at row offset off within the core's
    1024-row block. No collectives."""
    bass, bacc, tile, mybir = _mybir()
    from concourse.masks import make_identity

    fp32 = mybir.dt.float32
    bf16 = mybir.dt.bfloat16
    u8 = mybir.dt.uint8
    i8 = mybir.dt.int8
    Alu = mybir.AluOpType
    Act = mybir.ActivationFunctionType
    MS = bass.MemorySpace

    njc = NJC
    nic = nb // 128
    np8 = NP8

    nc = bacc.Bacc(num_devices=NCORES)
    hsb_i = nc.declare_dram_parameter(
        "hsb_o", [128, njc * NH * (HID + 1)], bf16, isOutput=False)
    hpair_i = nc.declare_dram_parameter(
        "hpair_o", [128, njc * NH * HID], bf16, isOutput=False)
    small_i = nc.declare_dram_parameter(
        "small_o", [128, njc * NH * 2], fp32, isOutput=False)
    srow_i = nc.declare_dram_parameter(
        "srow_o", [1, NH * HID], fp32, isOutput=False)
    fown_i = nc.declare_dram_parameter(
        "fown_o", [NBLK, 2 * NH], fp32, isOutput=False)
    adjp_d = nc.declare_dram_parameter("adjp_blk", [nb, np8], u8,
                                       isOutput=False)
    out_d = nc.declare_dram_parameter(
        "out_blk", [nb + nic * 2, NH * HID], i8, isOutput=True)

    with tile.TileContext(nc) as tc:
        with (
            tc.tile_pool(name="const", bufs=1) as const,
            tc.tile_pool(name="ld", bufs=3) as ld,
            tc.tile_pool(name="anat", bufs=2) as anat,
            tc.tile_pool(name="dramp", bufs=1, space=MS.DRAM) as dramp,
            tc.tile_pool(name="mtp", bufs=6) as mtp,
            tc.tile_pool(name="dep", bufs=8) as dep,
            tc.tile_pool(name="esb", bufs=3) as esb,
        ):
            ident = const.tile([128, 128], fp32, name="ident", tag="ident")
            make_identity(nc, ident)
            ones_row = const.tile([1, 128], fp32, name="ones_row",
                                  tag="ones_row")
            nc.vector.memset(ones_row, 1.0)

            # ---- load j-side blobs into SBUF ----
            Hsb = const.tile([128, njc, NH, HID + 1], bf16, name="Hsb",
                             tag="Hsb")
            nc.sync.dma_start(
                out=Hsb.rearrange("p a b c -> p (a b c)"), in_=hsb_i[:, :])
            Hpair = const.tile([128, njc, NH * HID], bf16, name="Hpair",
                               tag="Hpair")
            nc.sync.dma_start(
                out=Hpair.rearrange("p a b -> p (a b)"), in_=hpair_i[:, :])
            small = const.tile([128, njc * NH * 2], fp32,
                               name="small", tag="small")
            nc.sync.dma_start(out=small, in_=small_i[:, :])
            ETc = small[:, 0:njc * NH].rearrange("p (a b) -> p a b", a=njc)
            Vc = small[:, njc * NH:2 * njc * NH].rearrange(
                "p (a b) -> p a b", a=njc)
            S_row = const.tile([1, NH * HID], fp32, name="S_row", tag="S_row")
            nc.sync.dma_start(out=S_row, in_=srow_i[:, :])
            Sb = []
            ppsum = tc.alloc_tile_pool(name="ppsum", bufs=2, space=MS.PSUM)
            for h in range(NH):
                sb_ps = ppsum.tile([128, HID], fp32, name="sb_ps", tag="ps")
                nc.tensor.matmul(sb_ps, ones_row,
                                 S_row[:, h * HID:(h + 1) * HID])
                t = const.tile([128, HID], fp32, name=f"Sb{h}", tag=f"Sb{h}")
                nc.scalar.copy(out=t, in_=sb_ps)
                Sb.append(t)

            # ---- stage A: bit-unpack mask (+ deg) ----
            madj = dramp.tile([nb, n], bf16, name="madj", tag="madj")
            deg_sb = const.tile([128, nic], fp32, name="deg_sb", tag="deg_sb")
            for ics in range(nic):
                r0 = ics * 128
                v = [anat.tile([128, np8], fp32, name=f"v{i}", tag=f"v{i}")
                     for i in range(2)]
                nc.gpsimd.dma_start(out=v[0], in_=adjp_d[r0:r0 + 128, :])
                dacc = anat.tile([128, 8], fp32, name="dacc", tag="dacc")
                for s, b in enumerate(range(7, -1, -1)):
                    pbf = anat.tile([128, np8], bf16, name=f"pbf{b}", tag="pbf")
                    nc.vector.tensor_scalar(
                        out=pbf, in0=v[s % 2],
                        scalar1=float(1 << b), scalar2=0.0,
                        op0=Alu.is_ge, op1=Alu.add,
                        accum_out=dacc[:, b:b + 1])
                    if b > 0:
                        nc.vector.scalar_tensor_tensor(
                            out=v[(s + 1) % 2], in0=pbf,
                            scalar=-float(1 << b), in1=v[s % 2],
                            op0=Alu.mult, op1=Alu.add)
                    nc.sync.dma_start(
                        out=madj[ics * 128:(ics + 1) * 128,
                                 b * np8:(b + 1) * np8],
                        in_=pbf)
                nc.vector.tensor_reduce(
                    deg_sb[:, ics:ics + 1], dacc,
                    mybir.AxisListType.X, Alu.add)

            # ---- i-side prologue: host-exact F rows for this chunk ----
            # FrowT [128, nic, 8] loads directly (partition dim = row)
            FrowT = const.tile([128, nic, 2 * NH], fp32, name="FrowT",
                               tag="FrowT")
            for g in range(nic):
                nc.sync.dma_start(
                    out=FrowT[:, g, :],
                    in_=fown_i[off + g * 128:off + (g + 1) * 128, :])

            # Frow [8, nb] via per-group transposes of FrowT
            Frow = const.tile([2 * NH, nb], fp32, name="Frow", tag="Frow")
            for g in range(nic):
                fr_ps = ppsum.tile([2 * NH, 128], fp32, name="fr_ps", tag="ps")
                nc.tensor.matmul(fr_ps, FrowT[:, g, :], ident)
                nc.scalar.copy(out=Frow[:, g * 128:(g + 1) * 128], in_=fr_ps)
            ppsum.release()

            # row side: R8 = e^{0.8 Frow}; U'T = e^{0.2 FrowT}
            R8 = const.tile([2 * NH, nb], fp32, name="R8", tag="R8")
            nc.scalar.activation(R8, Frow, Act.Exp, scale=0.8)
            UpT = const.tile([128, nic, 2 * NH], fp32, name="UpT", tag="UpT")
            nc.scalar.activation(
                UpT.rearrange("p a b -> p (a b)"),
                FrowT.rearrange("p a b -> p (a b)"), Act.Exp, scale=0.2)

            # r broadcast per head [128, nb] bf16 via DRAM bounce
            r8_dram = dramp.tile([2 * NH, nb], fp32, name="r8_dram", tag="r8d")
            nc.sync.dma_start(out=r8_dram, in_=R8)
            rbc = []
            for h in range(NH):
                t = const.tile([128, nb], bf16, name=f"rbc{h}", tag=f"rbc{h}")
                srow = r8_dram[2 * h:2 * h + 1, :]
                src_b = bass.AP(
                    tensor=srow.tensor, offset=srow.offset,
                    ap=[[0, 128]] + [list(d) for d in srow.ap[1:]])
                nc.gpsimd.dma_start(out=t, in_=src_b)
                rbc.append(t)

            degbar = const.tile([128, nic], fp32, name="degbar", tag="degbar")
            nc.vector.tensor_scalar(
                out=degbar, in0=deg_sb, scalar1=-1.0, scalar2=float(n),
                op0=Alu.mult, op1=Alu.add)

            # ---------------- main loop ----------------
            with (
                tc.tile_pool(name="mm", bufs=1, space=MS.PSUM) as mm,
                tc.tile_pool(name="ep", bufs=2, space=MS.PSUM) as ep,
            ):
                X = [mm.tile([HID + 1, nb], fp32, name=f"X{h}", tag=f"X{h}")
                     for h in range(NH)]
                HM = [mm.tile([128, nb], fp32, name=f"HM{p}", tag=f"HM{p}")
                      for p in range(2)]
                for jc in range(njc):
                    mT = mtp.tile([128, nb], bf16, name="mT", tag="mT")
                    nc.sync.dma_start_transpose(
                        out=mT, in_=madj[:, jc * 128:(jc + 1) * 128])
                    for h in range(NH):
                        D2 = dep.tile([128, nb], bf16, name="D2", tag="D2")
                        nc.vector.tensor_scalar(
                            out=D2, in0=rbc[h],
                            scalar1=ETc[:, jc, h:h + 1],
                            scalar2=Vc[:, jc, h:h + 1],
                            op0=Alu.mult, op1=Alu.max)
                        E2 = dep.tile([128, nb], bf16, name="E2", tag="E2")
                        eng_tt = nc.gpsimd if h >= 2 else nc.vector
                        eng_tt.tensor_mul(E2, mT, D2)
                        nc.tensor.matmul(
                            X[h], Hsb[:, jc, h, :], E2,
                            start=(jc == 0), stop=(jc == njc - 1))
                    for p in range(2):
                        nc.tensor.matmul(
                            HM[p],
                            Hpair[:, jc, 128 * p:128 * (p + 1)], mT,
                            start=(jc == 0), stop=(jc == njc - 1))

                # ---------------- epilogue ----------------
                XS = []
                for h in range(NH):
                    t = esb.tile([HID + 1, nb], fp32,
                                 name=f"XS{h}", tag=f"XS{h}", bufs=1)
                    nc.scalar.copy(out=t, in_=X[h])
                    XS.append(t)
                HMS = []
                for p in range(2):
                    t = esb.tile([128, nb], fp32,
                                 name=f"HMS{p}", tag=f"HMS{p}", bufs=1)
                    nc.scalar.copy(out=t, in_=HM[p])
                    HMS.append(t)

                for g in range(nic):
                    sl = slice(g * 128, (g + 1) * 128)
                    hmT = []
                    for p in range(2):
                        tp = ep.tile([128, 128], fp32, name="tp", tag="tp")
                        nc.tensor.matmul(tp, HMS[p][:, sl], ident)
                        t = esb.tile([128, 128], fp32,
                                     name=f"hmT{p}", tag=f"hmT{p}", bufs=2)
                        nc.scalar.copy(out=t, in_=tp)
                        hmT.append(t)
                    out_tile = esb.tile([128, NH * HID], fp32,
                                        name="out_tile", tag="otile", bufs=2)
                    for h in range(NH):
                        tp = ep.tile([128, HID + 1], fp32, name="tpx", tag="tp")
                        nc.tensor.matmul(
                            tp, XS[h][:, sl], ident[0:HID + 1, 0:HID + 1])
                        XT = esb.tile([128, HID + 1], fp32, name="XT", tag="XT")
                        nc.scalar.copy(out=XT, in_=tp)
                        upc = UpT[:, g, 2 * h:2 * h + 1]
                        n1 = esb.tile([128, HID], fp32, name="n1", tag="n1")
                        nc.vector.tensor_scalar(
                            out=n1, in0=XT[:, 0:HID], scalar1=upc,
                            scalar2=None, op0=Alu.mult)
                        n2 = esb.tile([128, HID], fp32, name="n2", tag="n2")
                        nc.vector.scalar_tensor_tensor(
                            out=n2,
                            in0=hmT[h // 2][:, (h % 2) * HID:
                                            (h % 2) * HID + HID],
                            scalar=-1.0, in1=n1, op0=Alu.mult, op1=Alu.add)
                        n3 = esb.tile([128, HID], fp32, name="n3", tag="n3")
                        nc.vector.tensor_add(n3, n2, Sb[h])
                        dcol = esb.tile([128, 1], fp32, name="dcol", tag="dcol")
                        nc.vector.tensor_scalar(
                            out=dcol, in0=XT[:, HID:HID + 1], scalar1=upc,
                            scalar2=degbar[:, g:g + 1],
                            op0=Alu.mult, op1=Alu.add)
                        rec = esb.tile([128, 1], fp32, name="rec", tag="rec")
                        nc.vector.reciprocal(rec, dcol)
                        smT = esb.tile([128, HID], fp32, name="smT", tag="smT")
                        nc.vector.tensor_scalar(
                            out=smT, in0=n3, scalar1=rec, scalar2=None,
                            op0=Alu.mult)
                        # elu = (max(sm,0)-1) + exp(min(sm,0))
                        ea = esb.tile([128, HID], fp32, name="ea", tag="ea")
                        nc.vector.tensor_scalar_min(ea, smT, 0.0)
                        eb = esb.tile([128, HID], fp32, name="eb", tag="eb")
                        nc.scalar.activation(eb, ea, Act.Exp)
                        ec = esb.tile([128, HID], fp32, name="ec", tag="ec")
                        nc.vector.tensor_scalar(
                            out=ec, in0=smT, scalar1=0.0, scalar2=-1.0,
                            op0=Alu.max, op1=Alu.add)
                        nc.vector.tensor_add(
                            out_tile[:, h * HID:(h + 1) * HID], eb, ec)
                    # int8 quantize with per-row scale (host dequantizes)
                    rhi = esb.tile([128, 1], fp32, name="rhi", tag="rhi")
                    nc.vector.tensor_reduce(
                        rhi, out_tile, mybir.AxisListType.X, Alu.max)
                    rlo = esb.tile([128, 1], fp32, name="rlo", tag="rlo")
                    nc.vector.tensor_reduce(
                        rlo, out_tile, mybir.AxisListType.X, Alu.min)
                    rneg = esb.tile([128, 1], fp32, name="rneg", tag="rneg")
                    nc.vector.tensor_scalar(
                        out=rneg, in0=rlo, scalar1=-1.0, scalar2=None,
                        op0=Alu.mult)
                    rabs = esb.tile([128, 1], fp32, name="rabs", tag="rabs")
                    nc.vector.tensor_tensor(rabs, rhi, rneg, Alu.max)
                    rsc = esb.tile([128, 1], fp32, name="rsc", tag="rsc")
                    nc.vector.tensor_scalar(
                        out=rsc, in0=rabs, scalar1=1e-30,
                        scalar2=1.0 / 127.0, op0=Alu.max, op1=Alu.mult)
                    rinv = esb.tile([128, 1], fp32, name="rinv", tag="rinv")
                    nc.vector.reciprocal(rinv, rsc)
                    qt = esb.tile([128, NH * HID], i8,
                                  name="qt", tag="qt", bufs=2)
                    nc.vector.tensor_scalar(
                        out=qt, in0=out_tile, scalar1=rinv, scalar2=None,
                        op0=Alu.mult)
                    nc.sync.dma_start(
                        out=out_d[g * 128:(g + 1) * 128, :], in_=qt)
                    nc.sync.dma_start(
                        out=out_d[nb + 2 * g:nb + 2 * g + 2, :].bitcast(fp32),
                        in_=rsc)
    nc.compile()
    return nc


def _make_dispatcher(nc, mesh, mybir, bass2jax, jax):
    from jax.experimental.shard_map import shard_map
    from jax.sharding import PartitionSpec

    partition_name = (nc.partition_id_tensor.name
                      if nc.partition_id_tensor else None)
    in_names, out_names, out_avals = [], [], []
    for alloc in nc.m.functions[0].allocations:
        if not isinstance(alloc, mybir.MemoryLocationSet):
            continue
        name = alloc.memorylocations[0].name
        if alloc.kind == "ExternalInput":
            if name != partition_name:
                in_names.append(name)
        elif alloc.kind == "ExternalOutput":
            out_names.append(name)
            out_avals.append(jax.core.ShapedArray(
                tuple(alloc.tensor_shape), mybir.dt.np(alloc.dtype)))
    n_params = len(in_names)
    n_outs = len(out_avals)
    all_names = in_names + out_names
    if partition_name is not None:
        all_names.append(partition_name)
    donate = tuple(range(n_params, n_params + n_outs))

    def _body(*args, _nc=nc, _avals=tuple(out_avals),
              _in=tuple(all_names), _out=tuple(out_names),
              _pn=partition_name):
        operands = list(args)
        if _pn is not None:
            operands.append(bass2jax.partition_id_tensor())
        outs = bass2jax._bass_exec_p.bind(
            *operands, out_avals=_avals, in_names=_in,
            out_names=_out, lowering_input_output_aliases=(),
            sim_require_finite=True, sim_require_nnan=True, nc=_nc)
        return tuple(outs)

    specs = (PartitionSpec("core"),) * (n_params + n_outs)
    out_specs = (PartitionSpec("core"),) * n_outs
    sharded = jax.jit(
        shard_map(_body, mesh=mesh, in_specs=specs, out_specs=out_specs,
                  check_rep=False),
        donate_argnums=donate, keep_unused=True)
    zshapes = [((NCORES * av.shape[0],) + tuple(av.shape[1:]), av.dtype)
               for av in out_avals]
    return sharded, in_names, out_names, zshapes


def _get_runner():
    if "runner" in _cache:
        return _cache["runner"]
    import jax
    import jax.numpy as jnp
    from jax.sharding import Mesh, PartitionSpec, NamedSharding
    from concourse import bass2jax, mybir

    bass2jax.install_neuronx_cc_hook()
    devices = jax.devices()[:NCORES]
    mesh = Mesh(np.asarray(devices), ("core",))
    shard8 = NamedSharding(mesh, PartitionSpec("core"))

    prog_a = _make_dispatcher(build_prologue(), mesh, mybir, bass2jax, jax)
    progs_b = []
    chunk_cache = {}
    for nb, off in CHUNKS:
        key = (nb, off)
        chunk_cache[key] = _make_dispatcher(
            build_chunk(nb, off), mesh, mybir, bass2jax, jax)
        progs_b.append(chunk_cache[key])

    # one device-side zeros dispatch for ALL donated output buffers
    zspecs = list(prog_a[3])
    boundaries = [len(zspecs)]
    for pb in progs_b:
        zspecs.extend(pb[3])
        boundaries.append(len(zspecs))

    def _zeros_all():
        return tuple(jnp.zeros(s, d) for s, d in zspecs)

    zeros_all_jit = jax.jit(
        _zeros_all, out_shardings=(shard8,) * len(zspecs))

    runner = (prog_a, progs_b, boundaries, zeros_all_jit, devices, shard8)
    _cache["runner"] = runner
    return runner


def _run_once(x, adj, W, a, jax, ml_dtypes):
    prog_a, progs_b, boundaries, zeros_all_jit, devices, sh8 = _get_runner()
    pack, quant = _get_packer()

    if "xwa_buf" not in _cache:
        _cache["xwa_buf"] = np.empty((NCORES, SEG), np.uint8)
        _cache["adj_bufs"] = [
            np.empty((NCORES * nb, NP8), np.uint8) for nb, _ in CHUNKS]

    # ---- exact logits F = x @ Q (tiny GEMM) + per-column int8 x ----
    # Q[2h] = W_h a1_h, Q[2h+1] = W_h a2_h; exp(f) on device then uses
    # EXACT f32 logits — more accurate than deriving f from bf16 x.
    Q = np.empty((2 * NH, EMB), np.float32)
    for h in range(NH):
        Q[2 * h] = W[h] @ a[h][:HID, 0]
        Q[2 * h + 1] = W[h] @ a[h][HID:, 0]
    F = x @ Q.T                                   # [N, 8] f32
    xu8, scale = quant(x)                         # biased u8, per-col scale
    csb = np.empty((128, 2), np.float32)
    csb[:, 0] = scale
    csb[:, 1] = -128.0 * scale                    # bias: (u8 - 128) * scale

    # ---- xwa: [x u8 | colscale/bias | F rows | W eighth ] per core ----
    xwa = _cache["xwa_buf"]
    W_bytes = W.astype(ml_dtypes.bfloat16).reshape(-1).view(np.uint8)
    xwa[:, :XB] = xu8.reshape(NCORES, -1)
    xwa[:, OFF_CS:OFF_F] = csb.reshape(-1).view(np.uint8)[None, :]
    xwa[:, OFF_F:OFF_W] = F.reshape(NCORES, -1).view(np.uint8)
    xwa[:, OFF_W:] = W_bytes.reshape(NCORES, WB8)
    xwad = jax.device_put(xwa.reshape(-1), sh8)
    zs_all = zeros_all_jit()

    # ---- progA: j-side precompute (overlaps adj packing below) ----
    sharded_a, in_a, out_a, _ = prog_a
    a_outs = sharded_a(xwad, *zs_all[:boundaries[0]])
    blob_by_name = dict(zip(out_a, a_outs))

    # Pack ALL adj chunks now, while the (2.1MB) xwa upload drains the
    # wire — keeps the C packer off the CPU during the adj upload phase
    # (the relay shares this host's single core).
    for c, (nb, off) in enumerate(CHUNKS):
        buf = _cache["adj_bufs"][c]
        for core in range(NCORES):
            pack(adj, core * NBLK + off, nb, buf[core * nb:(core + 1) * nb])

    # ---- chunk pipeline: put -> dispatch -> async fetch ----
    outs = []
    for c, (nb, off) in enumerate(CHUNKS):
        adjp = jax.device_put(_cache["adj_bufs"][c], sh8)
        sharded_b, in_b, out_b, _ = progs_b[c]
        vals = dict(blob_by_name)
        vals["adjp_blk"] = adjp
        zs = zs_all[boundaries[c]:boundaries[c + 1]]
        out_arrs = sharded_b(*[vals[nm] for nm in in_b], *zs)
        out_arrs[0].copy_to_host_async()
        outs.append(out_arrs[0])

    # ---- collect + dequantize ----
    res = np.empty((NCORES, NBLK, NH * HID), np.float32)
    for c, (nb, off) in enumerate(CHUNKS):
        nbp = nb + (nb // 128) * 2
        arr = np.asarray(outs[c]).reshape(NCORES, nbp, NH * HID)
        q = arr[:, :nb, :].astype(np.float32)
        sc = np.ascontiguousarray(arr[:, nb:, :]).view(np.float32)
        np.multiply(q, sc.reshape(NCORES, nb, 1), out=q)
        res[:, off:off + nb] = q
    return res.reshape(N, NH * HID)


def kernel(x, adj, W, a):
    import sys
    for p in ("/opt/trn_rl_repo", "/opt/trn_rl_repo/concourse"):
        if p not in sys.path:
            sys.path.insert(0, p)

    import jax
    import ml_dtypes

    x = np.ascontiguousarray(np.asarray(x, dtype=np.float32))
    adj = np.ascontiguousarray(np.asarray(adj, dtype=np.int32))
    W = np.ascontiguousarray(np.asarray(W, dtype=np.float32))
    a = np.ascontiguousarray(np.asarray(a, dtype=np.float32))

    try:
        return _run_once(x, adj, W, a, jax, ml_dtypes)
    except Exception:
        # transient axon/NRT hiccup — retry from scratch
        import time
        time.sleep(0.5)
        try:
            return _run_once(x, adj, W, a, jax, ml_dtypes)
        except Exception:
            time.sleep(2.0)
            return _run_once(x, adj, W, a, jax, ml_dtypes)


# revision 33
# speedup vs baseline: 1.0103x; 1.0103x over previous
# GAT (graph attention) Trainium2 kernel — 8-core row-parallel SPMD,
# pipelined: one prologue program + row-chunk programs overlapping
# upload / exec / download on the slow axon tunnel.
#
# Math (per head h, rows I owned by a core):
#   h = x @ W_h ; f1 = h@a1 ; f2 = h@a2 ; z_ij = f1_i + f2_j
#   P_ij = adj_ij ? exp(lrelu(z)) : exp(9e-15 ~= 0) ; att = softmax_j(P)
#   out = elu( (P @ h) / (P @ 1) )
# Device factorization (avoids O(N^2) transcendentals):
#   exp(lrelu(z)) = u'_i * v'_j * max(r_i * w_j, 1)
#     r = e^{0.8 f1}, w = e^{0.8 f2}, u' = e^{0.2 f1}, v' = e^{0.2 f2}
#   E2[j,i] = m^T[j,i] * max(r_i * (w_j v'_j), v'_j)     (ts_dual + tt mult)
#   numer[d,i] = u'_i * ([h|1]^T E2)[d,i] + S[d] - (h^T m^T)[d,i]
#   denom[i]   = u'_i * Y1[i] + N - deg_i
#
# The e2e bottleneck is the axon RPC tunnel: ~80ms RTT, ~25-45MB/s shared
# bandwidth, one host CPU. Design:
#   - adj ships BIT-PACKED along j (uint8, 32x less wire), packed by a
#     gcc-compiled AVX2 helper (~22ms for all 256MB vs ~174ms numpy).
#   - x (bf16), W (bf16), a (f32) ship once as per-core slices in one
#     combined buffer (~2.1MB); progA all-gathers x and precomputes every
#     j-side quantity (h~, e^{f2} factors) into device-DRAM blobs.
#   - the row space is split into row-chunks; each chunk is a light progB
#     variant (row offset baked in, no collectives) dispatched as soon as
#     its adj chunk is on the wire. Chunk execs and int8 output downloads
#     overlap the upload of later chunks; only the last chunk's small
#     exec+fetch tail is exposed.
#   - output returns int8 with per-row f32 scales bitcast into tail rows;
#     host dequantizes to f32.

import ctypes
import os
import subprocess
import tempfile

import numpy as np

N = 8192
EMB = 128
HID = 64
NH = 4
NCORES = 8
NBLK = N // NCORES        # 1024 rows per core
NP8 = N // 8              # packed bytes per adj row
NJC = N // 128            # j chunks

# (rows, row offset) per chunk, per core; rows must be multiples of 128.
# Descending sizes: the last chunk's exec + download is the exposed tail.
CHUNKS = [(384, 0), (256, 384), (256, 640), (128, 896)]

# Per-core xwa segment layout:
#   [ x rows u8 (int8+128, per-col scale) | colscale/bias f32 [128,2]
#   | F rows f32 [NBLK,8] (f1/f2 per head, computed EXACTLY on host)
#   | W eighth bf16 | a eighth f32 ]
# x int8 + exact F is both SMALLER (169KB vs 264KB per core) and MORE
# ACCURATE than bf16 x: the error-critical exp(f) path uses exact f32
# logits, and the int8 h error washes out in the softmax average.
XB = NBLK * EMB           # x rows, 1 byte each
CSB = 128 * 2 * 4         # per-column scale+bias f32
FB = NBLK * 2 * NH * 4    # F rows f32
WB8 = NH * EMB * HID * 2 // NCORES
SEG = XB + CSB + FB + WB8
OFF_CS = XB
OFF_F = OFF_CS + CSB
OFF_W = OFF_F + FB

_cache = {}

_PACK_C = r"""
#include <stdint.h>
#include <stddef.h>
#if defined(__AVX2__)
#include <immintrin.h>
#endif

void pack_adj(const int32_t* restrict adj, uint8_t* restrict out,
              size_t nrows, size_t ncols) {
    size_t nb = ncols / 8;
    for (size_t r = 0; r < nrows; r++) {
        const int32_t* row = adj + r * ncols;
        uint8_t* orow = out + r * nb;
        size_t c = 0;
#if defined(__AVX2__)
        for (; c + 8 <= ncols; c += 8) {
            __m256i v = _mm256_loadu_si256((const __m256i*)(row + c));
            v = _mm256_slli_epi32(v, 31);
            int m = _mm256_movemask_ps(_mm256_castsi256_ps(v));
            orow[c / 8] = (uint8_t)m;
        }
#endif
        for (; c < ncols; c += 8) {
            uint8_t b = 0;
            for (int k = 0; k < 8; k++)
                b |= (uint8_t)((row[c + k] & 1) << k);
            orow[c / 8] = b;
        }
    }
}

/* column-wise absmax of a row-major [nrows, 128] f32 matrix */
void colmax_abs(const float* restrict x, float* restrict cmax,
                size_t nrows) {
    for (int e = 0; e < 128; e++) cmax[e] = 0.0f;
#if defined(__AVX2__)
    __m256 acc[16];
    for (int v = 0; v < 16; v++) acc[v] = _mm256_setzero_ps();
    const __m256 signmask = _mm256_castsi256_ps(_mm256_set1_epi32(0x7fffffff));
    for (size_t r = 0; r < nrows; r++) {
        const float* row = x + r * 128;
        for (int v = 0; v < 16; v++) {
            __m256 d = _mm256_and_ps(_mm256_loadu_ps(row + v * 8), signmask);
            acc[v] = _mm256_max_ps(acc[v], d);
        }
    }
    for (int v = 0; v < 16; v++) _mm256_storeu_ps(cmax + v * 8, acc[v]);
#else
    for (size_t r = 0; r < nrows; r++)
        for (int e = 0; e < 128; e++) {
            float d = x[r * 128 + e];
            if (d < 0) d = -d;
            if (d > cmax[e]) cmax[e] = d;
        }
#endif
}

/* quantize row-major [nrows, 128] f32 -> biased u8 with per-col 1/scale */
void quant_x(const float* restrict x, const float* restrict invscale,
             uint8_t* restrict out, size_t nrows) {
#if defined(__AVX2__)
    __m256 inv[16];
    for (int v = 0; v < 16; v++) inv[v] = _mm256_loadu_ps(invscale + v * 8);
    const __m256i bias = _mm256_set1_epi32(128);
    for (size_t r = 0; r < nrows; r++) {
        const float* row = x + r * 128;
        uint8_t* orow = out + r * 128;
        for (int v = 0; v < 16; v += 4) {
            __m256i q0 = _mm256_add_epi32(_mm256_cvtps_epi32(
                _mm256_mul_ps(_mm256_loadu_ps(row + v * 8), inv[v])), bias);
            __m256i q1 = _mm256_add_epi32(_mm256_cvtps_epi32(
                _mm256_mul_ps(_mm256_loadu_ps(row + v * 8 + 8), inv[v + 1])), bias);
            __m256i q2 = _mm256_add_epi32(_mm256_cvtps_epi32(
                _mm256_mul_ps(_mm256_loadu_ps(row + v * 8 + 16), inv[v + 2])), bias);
            __m256i q3 = _mm256_add_epi32(_mm256_cvtps_epi32(
                _mm256_mul_ps(_mm256_loadu_ps(row + v * 8 + 24), inv[v + 3])), bias);
            /* 32 -> 16 -> 8 with saturation; repair 128-bit lane order */
            __m256i p01 = _mm256_packs_epi32(q0, q1);
            __m256i p23 = _mm256_packs_epi32(q2, q3);
            __m256i p = _mm256_packus_epi16(p01, p23);
            p = _mm256_permutevar8x32_epi32(
                p, _mm256_setr_epi32(0, 4, 1, 5, 2, 6, 3, 7));
            _mm256_storeu_si256((__m256i*)(orow + v * 8), p);
        }
    }
#else
    for (size_t r = 0; r < nrows; r++)
        for (int e = 0; e < 128; e++) {
            float q = x[r * 128 + e] * invscale[e];
            long iq = (long)(q >= 0 ? q + 0.5f : q - 0.5f) + 128;
            if (iq < 0) iq = 0;
            if (iq > 255) iq = 255;
            out[r * 128 + e] = (uint8_t)iq;
        }
#endif
}
"""


def _get_packer():
    """Compile the C bit-packer once; fall back to numpy packbits."""
    if "packer" in _cache:
        return _cache["packer"]
    try:
        d = os.path.join(tempfile.gettempdir(), "gat_pack_v2")
        so = os.path.join(d, "pack.so")
        if not os.path.exists(so):
            os.makedirs(d, exist_ok=True)
            src = os.path.join(d, "pack.c")
            with open(src, "w") as f:
                f.write(_PACK_C)
            subprocess.run(
                ["gcc", "-O3", "-march=native", "-shared", "-fPIC",
                 "-o", so + ".tmp", src],
                check=True, capture_output=True)
            os.replace(so + ".tmp", so)
        lib = ctypes.CDLL(so)
        lib.pack_adj.argtypes = [ctypes.c_void_p, ctypes.c_void_p,
                                 ctypes.c_size_t, ctypes.c_size_t]
        lib.pack_adj.restype = None
        lib.colmax_abs.argtypes = [ctypes.c_void_p, ctypes.c_void_p,
                                   ctypes.c_size_t]
        lib.colmax_abs.restype = None
        lib.quant_x.argtypes = [ctypes.c_void_p, ctypes.c_void_p,
                                ctypes.c_void_p, ctypes.c_size_t]
        lib.quant_x.restype = None

        # sanity checks against numpy
        test = np.random.default_rng(0).integers(0, 2, (4, 64), dtype=np.int32)
        got = np.empty((4, 8), np.uint8)
        lib.pack_adj(test.ctypes.data, got.ctypes.data, 4, 64)
        ref = np.packbits(test.astype(np.uint8), axis=1, bitorder="little")
        if not np.array_equal(got, ref):
            raise RuntimeError("C packer mismatch")
        tx = np.random.default_rng(1).normal(size=(16, 128)).astype(np.float32)
        cm = np.empty(128, np.float32)
        lib.colmax_abs(tx.ctypes.data, cm.ctypes.data, 16)
        if not np.allclose(cm, np.abs(tx).max(axis=0)):
            raise RuntimeError("C colmax mismatch")
        inv = (127.0 / np.maximum(cm, 1e-30)).astype(np.float32)
        qu = np.empty((16, 128), np.uint8)
        lib.quant_x(tx.ctypes.data, inv.ctypes.data, qu.ctypes.data, 16)
        qref = (np.clip(np.rint(tx * inv), -127, 127) + 128.0).astype(np.uint8)
        if np.abs(qu.astype(np.int32) - qref.astype(np.int32)).max() > 1:
            raise RuntimeError("C quant mismatch")

        def fn(adj_c, rowstart, nrows, dst):
            # adj_c: contiguous int32 [N, N]; dst: contiguous u8 [nrows, N/8]
            lib.pack_adj(adj_c.ctypes.data + rowstart * N * 4,
                         dst.ctypes.data, nrows, N)

        def quant(x_c):
            cmax = np.empty(128, np.float32)
            lib.colmax_abs(x_c.ctypes.data, cmax.ctypes.data, N)
            scale = np.maximum(cmax, 1e-30).astype(np.float32) / 127.0
            inv = (1.0 / scale).astype(np.float32)
            out = np.empty((N, EMB), np.uint8)
            lib.quant_x(x_c.ctypes.data, inv.ctypes.data, out.ctypes.data, N)
            return out, scale
    except Exception:
        def fn(adj_c, rowstart, nrows, dst):
            blk = adj_c.view(np.uint8)[rowstart:rowstart + nrows, ::4]
            dst[:] = np.packbits(blk, axis=1, bitorder="little")

        def quant(x_c):
            cmax = np.abs(x_c).max(axis=0)
            scale = np.maximum(cmax, 1e-30).astype(np.float32) / 127.0
            out = (np.clip(np.rint(x_c * (1.0 / scale)), -127, 127)
                   + 128.0).astype(np.uint8)
            return out, scale
    _cache["packer"] = (fn, quant)
    return _cache["packer"]


def _mybir():
    import concourse.bass as bass
    import concourse.bacc as bacc
    import concourse.tile as tile
    import concourse.mybir as mybir
    return bass, bacc, tile, mybir


def build_prologue(n=N):
    """progA: all-gather x/W/a, precompute all j-side quantities into
    device-DRAM blobs consumed by the chunk programs."""
    bass, bacc, tile, mybir = _mybir()
    from concourse.masks import make_identity

    fp32 = mybir.dt.float32
    bf16 = mybir.dt.bfloat16
    u8 = mybir.dt.uint8
    Alu = mybir.AluOpType
    Act = mybir.ActivationFunctionType
    MS = bass.MemorySpace

    njc = NJC
    nc = bacc.Bacc(num_devices=NCORES)
    xwa_d = nc.declare_dram_parameter("xwa_blk", [SEG], u8, isOutput=False)
    # outputs: j-side blobs (device-resident, never fetched to host)
    hsb_o = nc.declare_dram_parameter(
        "hsb_o", [128, njc * NH * (HID + 1)], bf16, isOutput=True)
    hpair_o = nc.declare_dram_parameter(
        "hpair_o", [128, njc * NH * HID], bf16, isOutput=True)
    # small_o packs [ETc | Vc] along the free dim
    small_o = nc.declare_dram_parameter(
        "small_o", [128, njc * NH * 2], fp32, isOutput=True)
    srow_o = nc.declare_dram_parameter(
        "srow_o", [1, NH * HID], fp32, isOutput=True)
    # own F rows passed through for the chunk programs' i-side
    fown_o = nc.declare_dram_parameter(
        "fown_o", [NBLK, 2 * NH], fp32, isOutput=True)

    with tile.TileContext(nc) as tc:
        with (
            tc.tile_pool(name="const", bufs=1) as const,
            tc.tile_pool(name="ld", bufs=3) as ld,
            tc.tile_pool(name="dramp", bufs=1, space=MS.DRAM) as dramp,
        ):
            ag_in = dramp.tile([SEG], u8, name="ag_in", tag="ag_in")
            ag_out = dramp.tile([NCORES * SEG], u8, name="ag_out", tag="ag_out",
                                addr_space="Shared")
            nc.sync.dma_start(out=ag_in, in_=xwa_d[:])
            nc.gpsimd.collective_compute(
                "AllGather", Alu.bypass,
                replica_groups=[list(range(NCORES))],
                ins=[ag_in], outs=[ag_out],
            )
            g2 = ag_out.rearrange("(c y) -> c y", c=NCORES)
            # biased-u8 x -> f32 (SWDGE cast); debias+scale happens after the
            # transpose (per-partition affine on xT)
            xag = dramp.tile([n, EMB], fp32, name="xag", tag="xag")
            nc.gpsimd.dma_start(out=xag, in_=g2[:, 0:XB])
            # gathered F rows [n, 8] f32 (row order = core-major = node id)
            Fag = dramp.tile([n, 2 * NH], fp32, name="Fag", tag="Fag")
            nc.sync.dma_start(out=Fag, in_=g2[:, OFF_F:OFF_F + FB].bitcast(fp32))
            # own F rows pass straight through to the chunk programs
            nc.sync.dma_start(
                out=fown_o[:, :],
                in_=xwa_d[OFF_F:OFF_F + FB].bitcast(fp32).rearrange(
                    "(r e) -> r e", e=2 * NH))
            Wg = dramp.tile([NH * EMB * HID], bf16, name="Wg", tag="Wg")
            nc.sync.dma_start(
                out=Wg, in_=g2[:, OFF_W:OFF_W + WB8].bitcast(bf16))
            W_v = Wg.rearrange("(h e d) -> e h d", h=NH, e=EMB)
            # permuted row view: row (b k) of xagv == original row 8k+b == j'
            xagv = xag.rearrange("(k b) e -> b k e", b=8)
            Fagv = Fag.rearrange("(k b) e -> b k e", b=8)

            ident = const.tile([128, 128], fp32, name="ident", tag="ident")
            make_identity(nc, ident)

            # per-column dequant affine: x = u8 * scale + bias
            csb = const.tile([128, 2], fp32, name="csb", tag="csb")
            nc.sync.dma_start(
                out=csb,
                in_=xwa_d[OFF_CS:OFF_CS + CSB].bitcast(fp32).rearrange(
                    "(p t) -> p t", t=2))

            ppsum = tc.alloc_tile_pool(name="ppsum", bufs=2, space=MS.PSUM)
            Wsb = const.tile([128, NH, HID], fp32, name="Wsb", tag="Wsb")
            nc.gpsimd.dma_start(out=Wsb, in_=W_v)

            # x_perm^T  [128e, n], dequantized per partition (= column e)
            xT = const.tile([128, n], fp32, name="xT", tag="xT")
            for jc in range(njc):
                b, m = jc // 8, jc % 8
                xt_nat = ld.tile([128, EMB], fp32, name="xt_nat", tag="xt_nat")
                nc.sync.dma_start(
                    out=xt_nat, in_=xagv[b, m * 128:(m + 1) * 128, :])
                ps = ppsum.tile([128, 128], fp32, name="ps", tag="ps")
                nc.tensor.matmul(ps, xt_nat, ident)
                nc.vector.tensor_scalar(
                    out=xT[:, jc * 128:(jc + 1) * 128], in0=ps,
                    scalar1=csb[:, 0:1], scalar2=csb[:, 1:2],
                    op0=Alu.mult, op1=Alu.add)

            xsum = const.tile([128, 1], fp32, name="xsum", tag="xsum")
            nc.vector.tensor_reduce(xsum, xT, mybir.AxisListType.X, Alu.add)

            Wflat = Wsb.rearrange("e h d -> e (h d)")

            # f columns for all j, loaded from the host-exact gathered F
            # (permuted row order j' like x)
            Fcol = const.tile([128, njc, 2 * NH], fp32, name="Fcol", tag="Fcol")
            for jc in range(njc):
                b, m = jc // 8, jc % 8
                nc.sync.dma_start(
                    out=Fcol[:, jc, :], in_=Fagv[b, m * 128:(m + 1) * 128, :])

            # scalar cols (j side): ETc = e^{f2} (= w v'), Vc = e^{0.2 f2}
            ETc = const.tile([128, njc, NH], fp32, name="ETc", tag="ETc")
            Vc = const.tile([128, njc, NH], fp32, name="Vc", tag="Vc")
            for h in range(NH):
                nc.scalar.activation(ETc[:, :, h], Fcol[:, :, 2 * h + 1], Act.Exp)
                nc.scalar.activation(
                    Vc[:, :, h], Fcol[:, :, 2 * h + 1], Act.Exp, scale=0.2)

            # H~ [128, njc, NH, HID+1] bf16 (ones col at [.., HID]) + pair
            Hsb = const.tile([128, njc, NH, HID + 1], bf16, name="Hsb", tag="Hsb")
            Hpair = const.tile([128, njc, NH * HID], bf16, name="Hpair",
                               tag="Hpair")
            nc.vector.memset(Hsb[:, :, :, HID], 1.0)
            for jc in range(njc):
                h_ps = ppsum.tile([128, NH, HID], fp32, name="h_ps", tag="ps")
                nc.tensor.matmul(
                    h_ps.rearrange("p h d -> p (h d)"),
                    xT[:, jc * 128:(jc + 1) * 128], Wflat)
                nc.scalar.copy(out=Hsb[:, jc, :, 0:HID], in_=h_ps)
                nc.scalar.copy(
                    out=Hpair[:, jc, :].rearrange("p (h d) -> p h d", h=NH),
                    in_=h_ps)

            # S row
            s_ps = ppsum.tile([1, NH * HID], fp32, name="s_ps", tag="ps")
            nc.tensor.matmul(s_ps, xsum, Wflat)
            S_sb = const.tile([1, NH * HID], fp32, name="S_sb", tag="S_sb")
            nc.scalar.copy(out=S_sb, in_=s_ps)
            nc.sync.dma_start(out=srow_o[:, :], in_=S_sb)
            ppsum.release()

            # write blobs
            nc.sync.dma_start(
                out=hsb_o[:, :], in_=Hsb.rearrange("p a b c -> p (a b c)"))
            nc.sync.dma_start(
                out=hpair_o[:, :], in_=Hpair.rearrange("p a b -> p (a b)"))
            nc.sync.dma_start(
                out=small_o[:, 0:njc * NH],
                in_=ETc.rearrange("p a b -> p (a b)"))
            nc.sync.dma_start(
                out=small_o[:, njc * NH:2 * njc * NH],
                in_=Vc.rearrange("p a b -> p (a b)"))
    nc.compile()
    return nc


def build_chunk(nb, off, n=N):
    """progB: i-side work for nb rows at row offset off within the core's
    1024-row block. No collectives."""
    bass, bacc, tile, mybir = _mybir()
    from concourse.masks import make_identity

    fp32 = mybir.dt.float32
    bf16 = mybir.dt.bfloat16
    u8 = mybir.dt.uint8
    i8 = mybir.dt.int8
    Alu = mybir.AluOpType
    Act = mybir.ActivationFunctionType
    MS = bass.MemorySpace

    njc = NJC
    nic = nb // 128
    np8 = NP8

    nc = bacc.Bacc(num_devices=NCORES)
    hsb_i = nc.declare_dram_parameter(
        "hsb_o", [128, njc * NH * (HID + 1)], bf16, isOutput=False)
    hpair_i = nc.declare_dram_parameter(
        "hpair_o", [128, njc * NH * HID], bf16, isOutput=False)
    small_i = nc.declare_dram_parameter(
        "small_o", [128, njc * NH * 2], fp32, isOutput=False)
    srow_i = nc.declare_dram_parameter(
        "srow_o", [1, NH * HID], fp32, isOutput=False)
    fown_i = nc.declare_dram_parameter(
        "fown_o", [NBLK, 2 * NH], fp32, isOutput=False)
    adjp_d = nc.declare_dram_parameter("adjp_blk", [nb, np8], u8,
                                       isOutput=False)
    out_d = nc.declare_dram_parameter(
        "out_blk", [nb + nic * 2, NH * HID], i8, isOutput=True)

    with tile.TileContext(nc) as tc:
        with (
            tc.tile_pool(name="const", bufs=1) as const,
            tc.tile_pool(name="ld", bufs=3) as ld,
            tc.tile_pool(name="anat", bufs=2) as anat,
            tc.tile_pool(name="dramp", bufs=1, space=MS.DRAM) as dramp,
            tc.tile_pool(name="mtp", bufs=6) as mtp,
            tc.tile_pool(name="dep", bufs=8) as dep,
            tc.tile_pool(name="esb", bufs=3) as esb,
        ):
            ident = const.tile([128, 128], fp32, name="ident", tag="ident")
            make_identity(nc, ident)
            ones_row = const.tile([1, 128], fp32, name="ones_row",
                                  tag="ones_row")
            nc.vector.memset(ones_row, 1.0)

            # ---- load j-side blobs into SBUF ----
            Hsb = const.tile([128, njc, NH, HID + 1], bf16, name="Hsb",
                             tag="Hsb")
            nc.sync.dma_start(
                out=Hsb.rearrange("p a b c -> p (a b c)"), in_=hsb_i[:, :])
            Hpair = const.tile([128, njc, NH * HID], bf16, name="Hpair",
                               tag="Hpair")
            nc.sync.dma_start(
                out=Hpair.rearrange("p a b -> p (a b)"), in_=hpair_i[:, :])
            small = const.tile([128, njc * NH * 2], fp32,
                               name="small", tag="small")
            nc.sync.dma_start(out=small, in_=small_i[:, :])
            ETc = small[:, 0:njc * NH].rearrange("p (a b) -> p a b", a=njc)
            Vc = small[:, njc * NH:2 * njc * NH].rearrange(
                "p (a b) -> p a b", a=njc)
            S_row = const.tile([1, NH * HID], fp32, name="S_row", tag="S_row")
            nc.sync.dma_start(out=S_row, in_=srow_i[:, :])
            Sb = []
            ppsum = tc.alloc_tile_pool(name="ppsum", bufs=2, space=MS.PSUM)
            for h in range(NH):
                sb_ps = ppsum.tile([128, HID], fp32, name="sb_ps", tag="ps")
                nc.tensor.matmul(sb_ps, ones_row,
                                 S_row[:, h * HID:(h + 1) * HID])
                t = const.tile([128, HID], fp32, name=f"Sb{h}", tag=f"Sb{h}")
                nc.scalar.copy(out=t, in_=sb_ps)
                Sb.append(t)

            # ---- stage A: bit-unpack mask (+ deg) ----
            madj = dramp.tile([nb, n], bf16, name="madj", tag="madj")
            deg_sb = const.tile([128, nic], fp32, name="deg_sb", tag="deg_sb")
            for ics in range(nic):
                r0 = ics * 128
                v = [anat.tile([128, np8], fp32, name=f"v{i}", tag=f"v{i}")
                     for i in range(2)]
                nc.gpsimd.dma_start(out=v[0], in_=adjp_d[r0:r0 + 128, :])
                dacc = anat.tile([128, 8], fp32, name="dacc", tag="dacc")
                for s, b in enumerate(range(7, -1, -1)):
                    pbf = anat.tile([128, np8], bf16, name=f"pbf{b}", tag="pbf")
                    nc.vector.tensor_scalar(
                        out=pbf, in0=v[s % 2],
                        scalar1=float(1 << b), scalar2=0.0,
                        op0=Alu.is_ge, op1=Alu.add,
                        accum_out=dacc[:, b:b + 1])
                    if b > 0:
                        nc.vector.scalar_tensor_tensor(
                            out=v[(s + 1) % 2], in0=pbf,
                            scalar=-float(1 << b), in1=v[s % 2],
                            op0=Alu.mult, op1=Alu.add)
                    nc.sync.dma_start(
                        out=madj[ics * 128:(ics + 1) * 128,
                                 b * np8:(b + 1) * np8],
                        in_=pbf)
                nc.vector.tensor_reduce(
                    deg_sb[:, ics:ics + 1], dacc,
                    mybir.AxisListType.X, Alu.add)

            # ---- i-side prologue: host-exact F rows for this chunk ----
            # FrowT [128, nic, 8] loads directly (partition dim = row)
            FrowT = const.tile([128, nic, 2 * NH], fp32, name="FrowT",
                               tag="FrowT")
            for g in range(nic):
                nc.sync.dma_start(
                    out=FrowT[:, g, :],
                    in_=fown_i[off + g * 128:off + (g + 1) * 128, :])

            # Frow [8, nb] via per-group transposes of FrowT
            Frow = const.tile([2 * NH, nb], fp32, name="Frow", tag="Frow")
            for g in range(nic):
                fr_ps = ppsum.tile([2 * NH, 128], fp32, name="fr_ps", tag="ps")
                nc.tensor.matmul(fr_ps, FrowT[:, g, :], ident)
                nc.scalar.copy(out=Frow[:, g * 128:(g + 1) * 128], in_=fr_ps)
            ppsum.release()

            # row side: R8 = e^{0.8 Frow}; U'T = e^{0.2 FrowT}
            R8 = const.tile([2 * NH, nb], fp32, name="R8", tag="R8")
            nc.scalar.activation(R8, Frow, Act.Exp, scale=0.8)
            UpT = const.tile([128, nic, 2 * NH], fp32, name="UpT", tag="UpT")
            nc.scalar.activation(
                UpT.rearrange("p a b -> p (a b)"),
                FrowT.rearrange("p a b -> p (a b)"), Act.Exp, scale=0.2)

            # r broadcast per head [128, nb] bf16 via DRAM bounce
            r8_dram = dramp.tile([2 * NH, nb], fp32, name="r8_dram", tag="r8d")
            nc.sync.dma_start(out=r8_dram, in_=R8)
            rbc = []
            for h in range(NH):
                t = const.tile([128, nb], bf16, name=f"rbc{h}", tag=f"rbc{h}")
                srow = r8_dram[2 * h:2 * h + 1, :]
                src_b = bass.AP(
                    tensor=srow.tensor, offset=srow.offset,
                    ap=[[0, 128]] + [list(d) for d in srow.ap[1:]])
                nc.gpsimd.dma_start(out=t, in_=src_b)
                rbc.append(t)

            degbar = const.tile([128, nic], fp32, name="degbar", tag="degbar")
            nc.vector.tensor_scalar(
                out=degbar, in0=deg_sb, scalar1=-1.0, scalar2=float(n),
                op0=Alu.mult, op1=Alu.add)

            # ---------------- main loop ----------------
            with (
                tc.tile_pool(name="mm", bufs=1, space=MS.PSUM) as mm,
                tc.tile_pool(name="ep", bufs=2, space=MS.PSUM) as ep,
            ):
                X = [mm.tile([HID + 1, nb], fp32, name=f"X{h}", tag=f"X{h}")
                     for h in range(NH)]
                HM = [mm.tile([128, nb], fp32, name=f"HM{p}", tag=f"HM{p}")
                      for p in range(2)]
                for jc in range(njc):
                    mT = mtp.tile([128, nb], bf16, name="mT", tag="mT")
                    nc.sync.dma_start_transpose(
                        out=mT, in_=madj[:, jc * 128:(jc + 1) * 128])
                    for h in range(NH):
                        D2 = dep.tile([128, nb], bf16, name="D2", tag="D2")
                        nc.vector.tensor_scalar(
                            out=D2, in0=rbc[h],
                            scalar1=ETc[:, jc, h:h + 1],
                            scalar2=Vc[:, jc, h:h + 1],
                            op0=Alu.mult, op1=Alu.max)
                        E2 = dep.tile([128, nb], bf16, name="E2", tag="E2")
                        eng_tt = nc.gpsimd if h >= 2 else nc.vector
                        eng_tt.tensor_mul(E2, mT, D2)
                        nc.tensor.matmul(
                            X[h], Hsb[:, jc, h, :], E2,
                            start=(jc == 0), stop=(jc == njc - 1))
                    for p in range(2):
                        nc.tensor.matmul(
                            HM[p],
                            Hpair[:, jc, 128 * p:128 * (p + 1)], mT,
                            start=(jc == 0), stop=(jc == njc - 1))

                # ---------------- epilogue ----------------
                XS = []
                for h in range(NH):
                    t = esb.tile([HID + 1, nb], fp32,
                                 name=f"XS{h}", tag=f"XS{h}", bufs=1)
                    nc.scalar.copy(out=t, in_=X[h])
                    XS.append(t)
                HMS = []
                for p in range(2):
                    t = esb.tile([128, nb], fp32,
                                 name=f"HMS{p}", tag=f"HMS{p}", bufs=1)
                    nc.scalar.copy(out=t, in_=HM[p])
                    HMS.append(t)

                for g in range(nic):
                    sl = slice(g * 128, (g + 1) * 128)
                    hmT = []
                    for p in range(2):
                        tp = ep.tile([128, 128], fp32, name="tp", tag="tp")
                        nc.tensor.matmul(tp, HMS[p][:, sl], ident)
                        t = esb.tile([128, 128], fp32,
                                     name=f"hmT{p}", tag=f"hmT{p}", bufs=2)
                        nc.scalar.copy(out=t, in_=tp)
                        hmT.append(t)
                    out_tile = esb.tile([128, NH * HID], fp32,
                                        name="out_tile", tag="otile", bufs=2)
                    for h in range(NH):
                        tp = ep.tile([128, HID + 1], fp32, name="tpx", tag="tp")
                        nc.tensor.matmul(
                            tp, XS[h][:, sl], ident[0:HID + 1, 0:HID + 1])
                        XT = esb.tile([128, HID + 1], fp32, name="XT", tag="XT")
                        nc.scalar.copy(out=XT, in_=tp)
                        upc = UpT[:, g, 2 * h:2 * h + 1]
                        n1 = esb.tile([128, HID], fp32, name="n1", tag="n1")
                        nc.vector.tensor_scalar(
                            out=n1, in0=XT[:, 0:HID], scalar1=upc,
                            scalar2=None, op0=Alu.mult)
                        n2 = esb.tile([128, HID], fp32, name="n2", tag="n2")
                        nc.vector.scalar_tensor_tensor(
                            out=n2,
                            in0=hmT[h // 2][:, (h % 2) * HID:
                                            (h % 2) * HID + HID],
                            scalar=-1.0, in1=n1, op0=Alu.mult, op1=Alu.add)
                        n3 = esb.tile([128, HID], fp32, name="n3", tag="n3")
                        nc.vector.tensor_add(n3, n2, Sb[h])
                        dcol = esb.tile([128, 1], fp32, name="dcol", tag="dcol")
                        nc.vector.tensor_scalar(
                            out=dcol, in0=XT[:, HID:HID + 1], scalar1=upc,
                            scalar2=degbar[:, g:g + 1],
                            op0=Alu.mult, op1=Alu.add)
                        rec = esb.tile([128, 1], fp32, name="rec", tag="rec")
                        nc.vector.reciprocal(rec, dcol)
                        smT = esb.tile([128, HID], fp32, name="smT", tag="smT")
                        nc.vector.tensor_scalar(
                            out=smT, in0=n3, scalar1=rec, scalar2=None,
                            op0=Alu.mult)
                        # elu = (max(sm,0)-1) + exp(min(sm,0))
                        ea = esb.tile([128, HID], fp32, name="ea", tag="ea")
                        nc.vector.tensor_scalar_min(ea, smT, 0.0)
                        eb = esb.tile([128, HID], fp32, name="eb", tag="eb")
                        nc.scalar.activation(eb, ea, Act.Exp)
                        ec = esb.tile([128, HID], fp32, name="ec", tag="ec")
                        nc.vector.tensor_scalar(
                            out=ec, in0=smT, scalar1=0.0, scalar2=-1.0,
                            op0=Alu.max, op1=Alu.add)
                        nc.vector.tensor_add(
                            out_tile[:, h * HID:(h + 1) * HID], eb, ec)
                    # int8 quantize with per-row scale (host dequantizes)
                    rhi = esb.tile([128, 1], fp32, name="rhi", tag="rhi")
                    nc.vector.tensor_reduce(
                        rhi, out_tile, mybir.AxisListType.X, Alu.max)
                    rlo = esb.tile([128, 1], fp32, name="rlo", tag="rlo")
                    nc.vector.tensor_reduce(
                        rlo, out_tile, mybir.AxisListType.X, Alu.min)
                    rneg = esb.tile([128, 1], fp32, name="rneg", tag="rneg")
                    nc.vector.tensor_scalar(
                        out=rneg, in0=rlo, scalar1=-1.0, scalar2=None,
                        op0=Alu.mult)
                    rabs = esb.tile([128, 1], fp32, name="rabs", tag="rabs")
                    nc.vector.tensor_tensor(rabs, rhi, rneg, Alu.max)
                    rsc = esb.tile([128, 1], fp32, name="rsc", tag="rsc")
                    nc.vector.tensor_scalar(
                        out=rsc, in0=rabs, scalar1=1e-30,
                        scalar2=1.0 / 127.0, op0=Alu.max, op1=Alu.mult)
                    rinv = esb.tile([128, 1], fp32, name="rinv", tag="rinv")
                    nc.vector.reciprocal(rinv, rsc)
                    qt = esb.tile([128, NH * HID], i8,
                                  name="qt", tag="qt", bufs=2)
                    nc.vector.tensor_scalar(
                        out=qt, in0=out_tile, scalar1=rinv, scalar2=None,
                        op0=Alu.mult)
                    nc.sync.dma_start(
                        out=out_d[g * 128:(g + 1) * 128, :], in_=qt)
                    nc.sync.dma_start(
                        out=out_d[nb + 2 * g:nb + 2 * g + 2, :].bitcast(fp32),
                        in_=rsc)
    nc.compile()
    return nc


def _make_dispatcher(nc, mesh, mybir, bass2jax, jax):
    from jax.experimental.shard_map import shard_map
    from jax.sharding import PartitionSpec

    partition_name = (nc.partition_id_tensor.name
                      if nc.partition_id_tensor else None)
    in_names, out_names, out_avals = [], [], []
    for alloc in nc.m.functions[0].allocations:
        if not isinstance(alloc, mybir.MemoryLocationSet):
            continue
        name = alloc.memorylocations[0].name
        if alloc.kind == "ExternalInput":
            if name != partition_name:
                in_names.append(name)
        elif alloc.kind == "ExternalOutput":
            out_names.append(name)
            out_avals.append(jax.core.ShapedArray(
                tuple(alloc.tensor_shape), mybir.dt.np(alloc.dtype)))
    n_params = len(in_names)
    n_outs = len(out_avals)
    all_names = in_names + out_names
    if partition_name is not None:
        all_names.append(partition_name)
    donate = tuple(range(n_params, n_params + n_outs))
    in_avals = {}
    for alloc in nc.m.functions[0].allocations:
        if not isinstance(alloc, mybir.MemoryLocationSet):
            continue
        name = alloc.memorylocations[0].name
        if alloc.kind == "ExternalInput" and name in in_names:
            in_avals[name] = (tuple(alloc.tensor_shape),
                              mybir.dt.np(alloc.dtype))

    def _body(*args, _nc=nc, _avals=tuple(out_avals),
              _in=tuple(all_names), _out=tuple(out_names),
              _pn=partition_name):
        operands = list(args)
        if _pn is not None:
            operands.append(bass2jax.partition_id_tensor())
        outs = bass2jax._bass_exec_p.bind(
            *operands, out_avals=_avals, in_names=_in,
            out_names=_out, lowering_input_output_aliases=(),
            sim_require_finite=True, sim_require_nnan=True, nc=_nc)
        return tuple(outs)

    specs = (PartitionSpec("core"),) * (n_params + n_outs)
    out_specs = (PartitionSpec("core"),) * n_outs
    sharded = jax.jit(
        shard_map(_body, mesh=mesh, in_specs=specs, out_specs=out_specs,
                  check_rep=False),
        donate_argnums=donate, keep_unused=True)
    zshapes = [((NCORES * av.shape[0],) + tuple(av.shape[1:]), av.dtype)
               for av in out_avals]
    # AOT-compile to skip per-call trace/cache-lookup overhead on the one
    # shared host CPU; fall back to the plain jit callable on any failure.
    try:
        sh8 = jax.sharding.NamedSharding(mesh, PartitionSpec("core"))
        structs = [jax.ShapeDtypeStruct(
            (NCORES * in_avals[nm][0][0],) + tuple(in_avals[nm][0][1:]),
            in_avals[nm][1], sharding=sh8) for nm in in_names]
        structs += [jax.ShapeDtypeStruct(s, d, sharding=sh8)
                    for s, d in zshapes]
        sharded = sharded.lower(*structs).compile()
    except Exception:
        pass
    return sharded, in_names, out_names, zshapes


def _get_runner():
    if "runner" in _cache:
        return _cache["runner"]
    import jax
    import jax.numpy as jnp
    from jax.sharding import Mesh, PartitionSpec, NamedSharding
    from concourse import bass2jax, mybir

    bass2jax.install_neuronx_cc_hook()
    devices = jax.devices()[:NCORES]
    mesh = Mesh(np.asarray(devices), ("core",))
    shard8 = NamedSharding(mesh, PartitionSpec("core"))

    prog_a = _make_dispatcher(build_prologue(), mesh, mybir, bass2jax, jax)
    progs_b = []
    chunk_cache = {}
    for nb, off in CHUNKS:
        key = (nb, off)
        chunk_cache[key] = _make_dispatcher(
            build_chunk(nb, off), mesh, mybir, bass2jax, jax)
        progs_b.append(chunk_cache[key])

    # one device-side zeros dispatch for ALL donated output buffers
    zspecs = list(prog_a[3])
    boundaries = [len(zspecs)]
    for pb in progs_b:
        zspecs.extend(pb[3])
        boundaries.append(len(zspecs))

    def _zeros_all():
        return tuple(jnp.zeros(s, d) for s, d in zspecs)

    zeros_all_jit = jax.jit(
        _zeros_all, out_shardings=(shard8,) * len(zspecs))
    try:
        zeros_all_jit = zeros_all_jit.lower().compile()
    except Exception:
        pass

    runner = (prog_a, progs_b, boundaries, zeros_all_jit, devices, shard8)
    _cache["runner"] = runner
    return runner


def _run_once(x, adj, W, a, jax, ml_dtypes):
    prog_a, progs_b, boundaries, zeros_all_jit, devices, sh8 = _get_runner()
    pack, quant = _get_packer()

    if "xwa_buf" not in _cache:
        _cache["xwa_buf"] = np.empty((NCORES, SEG), np.uint8)
        _cache["adj_bufs"] = [
            np.empty((NCORES * nb, NP8), np.uint8) for nb, _ in CHUNKS]

    # ---- exact logits F = x @ Q (tiny GEMM) + per-column int8 x ----
    # Q[2h] = W_h a1_h, Q[2h+1] = W_h a2_h; exp(f) on device then uses
    # EXACT f32 logits — more accurate than deriving f from bf16 x.
    Q = np.empty((2 * NH, EMB), np.float32)
    for h in range(NH):
        Q[2 * h] = W[h] @ a[h][:HID, 0]
        Q[2 * h + 1] = W[h] @ a[h][HID:, 0]
    F = x @ Q.T                                   # [N, 8] f32
    xu8, scale = quant(x)                         # biased u8, per-col scale
    csb = np.empty((128, 2), np.float32)
    csb[:, 0] = scale
    csb[:, 1] = -128.0 * scale                    # bias: (u8 - 128) * scale

    # ---- xwa: [x u8 | colscale/bias | F rows | W eighth ] per core ----
    xwa = _cache["xwa_buf"]
    W_bytes = W.astype(ml_dtypes.bfloat16).reshape(-1).view(np.uint8)
    xwa[:, :XB] = xu8.reshape(NCORES, -1)
    xwa[:, OFF_CS:OFF_F] = csb.reshape(-1).view(np.uint8)[None, :]
    xwa[:, OFF_F:OFF_W] = F.reshape(NCORES, -1).view(np.uint8)
    xwa[:, OFF_W:] = W_bytes.reshape(NCORES, WB8)
    xwad = jax.device_put(xwa.reshape(-1), sh8)
    zs_all = zeros_all_jit()

    # ---- progA: j-side precompute (overlaps adj packing below) ----
    sharded_a, in_a, out_a, _ = prog_a
    a_outs = sharded_a(xwad, *zs_all[:boundaries[0]])
    blob_by_name = dict(zip(out_a, a_outs))

    # Pack ALL adj chunks now, while the (2.1MB) xwa upload drains the
    # wire — keeps the C packer off the CPU during the adj upload phase
    # (the relay shares this host's single core).
    for c, (nb, off) in enumerate(CHUNKS):
        buf = _cache["adj_bufs"][c]
        for core in range(NCORES):
            pack(adj, core * NBLK + off, nb, buf[core * nb:(core + 1) * nb])

    # ---- chunk pipeline: put -> dispatch -> async fetch ----
    outs = []
    for c, (nb, off) in enumerate(CHUNKS):
        adjp = jax.device_put(_cache["adj_bufs"][c], sh8)
        sharded_b, in_b, out_b, _ = progs_b[c]
        vals = dict(blob_by_name)
        vals["adjp_blk"] = adjp
        zs = zs_all[boundaries[c]:boundaries[c + 1]]
        out_arrs = sharded_b(*[vals[nm] for nm in in_b], *zs)
        out_arrs[0].copy_to_host_async()
        outs.append(out_arrs[0])

    # ---- collect + dequantize ----
    res = np.empty((NCORES, NBLK, NH * HID), np.float32)
    for c, (nb, off) in enumerate(CHUNKS):
        nbp = nb + (nb // 128) * 2
        arr = np.asarray(outs[c]).reshape(NCORES, nbp, NH * HID)
        q = arr[:, :nb, :].astype(np.float32)
        sc = np.ascontiguousarray(arr[:, nb:, :]).view(np.float32)
        np.multiply(q, sc.reshape(NCORES, nb, 1), out=q)
        res[:, off:off + nb] = q
    return res.reshape(N, NH * HID)


def kernel(x, adj, W, a):
    import sys
    for p in ("/opt/trn_rl_repo", "/opt/trn_rl_repo/concourse"):
        if p not in sys.path:
            sys.path.insert(0, p)

    import jax
    import ml_dtypes

    x = np.ascontiguousarray(np.asarray(x, dtype=np.float32))
    adj = np.ascontiguousarray(np.asarray(adj, dtype=np.int32))
    W = np.ascontiguousarray(np.asarray(W, dtype=np.float32))
    a = np.ascontiguousarray(np.asarray(a, dtype=np.float32))

    try:
        return _run_once(x, adj, W, a, jax, ml_dtypes)
    except Exception:
        # transient axon/NRT hiccup — retry from scratch
        import time
        time.sleep(0.5)
        try:
            return _run_once(x, adj, W, a, jax, ml_dtypes)
        except Exception:
            time.sleep(2.0)
            return _run_once(x, adj, W, a, jax, ml_dtypes)
